# revision 35
# baseline (speedup 1.0000x reference)
"""CKSAAP embedding kernel for Trainium2 (8 NeuronCores, data-parallel over batch).

Strategy per (sequence, gap t):
    hist[d, bin] = sum_i vals_t[i, d] * onehot(idx_t[i])[bin]
computed as 16 accumulating PE matmuls (K=128 positions per chunk,
stationary = vals chunk [128, 64] fp16, moving = one-hot [128, 400] fp16,
accumulated fp32 in PSUM). One-hots are built on-chip from an iota row
compared against the per-position pair index (exact 0/1 in fp16):
  - VectorE: tensor_scalar(is_equal) with per-partition scalar
  - ScalarE: Abs(iota - idx) then Relu(1 - .) (exact for integer values)
vals_t = emb + shift(emb, t+1) built from partition-shifted SBUF copies
(DMA) + one VectorE add; the 0.5/(L-t-1) scale is folded into the final
PSUM->SBUF evacuation on ScalarE.

Host side: shards batch 256 -> 8 cores x 32 seqs, precasts emb to fp16 in
chunk-major layout, precomputes pair indices (seq*20 + shifted seq, -1 for
out-of-range tail), and transposes the device output [b,t,64,400] to the
reference layout [b,t,20,20,64].
"""

import numpy as np

from concourse import bacc, mybir
from concourse.bass_utils import run_bass_kernel_spmd
from concourse.tile import TileContext

NCORES = 8
B, L, D = 256, 2048, 64
NSEQ = B // NCORES  # 32 sequences per core
P = 128
NCH = L // P  # 16 position chunks per sequence
KP1 = 4  # gaps t = 0..3
NBINS = 400
F16 = mybir.dt.float16
F32 = mybir.dt.float32

# fraction pattern for one-hot engine choice: every act_mod-th one-hot goes
# to ScalarE (2 activation ops), the rest to VectorE (1 tensor_scalar op).
ACT_MOD = 6

# engine mix for one-hot builds, cycled: v=VectorE is_equal (1 op),
# a=ScalarE Abs+Relu (2 ops). HW-tuned: 1-in-6 on ScalarE balances DVE/ACT.
# (GpSimd 'g' one-hots measured pathologically slow on HW -- do not use.)
OH_PATTERN = "vvvvva"


def build_program(
    nseq=NSEQ,
    act_mod=None,
    repeat=1,
    oh_pattern=OH_PATTERN,
    colpack=True,
    hw_loop=1,
    out_dma="sync",
    tt_engine="split",
    oh_bufs=12,
    ps_bufs=2,
    emb_bufs=2,
    out_f16=False,
    fold_scale=True,
    evac_engine="scalar",
    dmov=False,
):
    """Position-major layout: partition p holds positions [16p, 16p+16);
    'chunk' c is the strided position set {16p + c}. The shifted operand
    e[i+s] is then a free-dim offset within embA for c < 16-s, and needs a
    single shift-by-one-partition copy (embA1) for the tail chunks.

    oh_pattern: string over {'v','a'} cycled across one-hot builds
    (VectorE is_equal / ScalarE abs+relu). Overrides act_mod when given.
    hw_loop>1 wraps the body in a hardware For_i (timing instrument).
    """
    import contextlib

    nc = bacc.Bacc()
    EXT = KP1 * D  # extension: first 4 chunk-cols of the next partition
    OUT_DT = F16 if out_f16 else F32
    emb16 = nc.declare_dram_parameter("emb16", [nseq, P, NCH * D], F16, False)
    # combined +idx / -idx, cols [0:64] = +idx, [64:128] = -idx
    idxc = nc.declare_dram_parameter("idxc", [nseq, P, 2 * KP1 * NCH], F32, False)
    iota = nc.declare_dram_parameter("iota", [P, NBINS], F16, False)
    # cols 0..KP1: per-gap folded scales 0.5/(L-t-1) broadcast down partitions;
    # cols KP1..KP1+2: pair-packed per-partition scales (row r -> gap 2p+r//64)
    scb = nc.declare_dram_parameter("scb", [P, KP1 + 2], F32, False)
    hist = nc.declare_dram_parameter("hist", [nseq, KP1 * D, NBINS], OUT_DT, True)

    def eng(name):
        return {
            "sync": nc.sync,
            "scalar": nc.scalar,
            "vector": nc.vector,
            "gpsimd": nc.gpsimd,
            "tensor": nc.tensor,
        }[name]

    with TileContext(nc) as tc:
        with (
            tc.tile_pool(name="const", bufs=1) as constp,
            tc.tile_pool(name="emb", bufs=emb_bufs) as embp,
            tc.tile_pool(name="oh", bufs=oh_bufs) as ohp,
            tc.tile_pool(name="ps", bufs=ps_bufs, space="PSUM") as psp,
            tc.tile_pool(name="outs", bufs=8) as outsp,
        ):
            iota_t = constp.tile([P, NBINS], F16)
            nc.sync.dma_start(out=iota_t[:], in_=iota[:])
            scb_t = constp.tile([P, KP1 + 2], F32)
            nc.sync.dma_start(out=scb_t[:], in_=scb[:])

            loop_cm = (
                tc.For_i(0, hw_loop, 1) if hw_loop > 1 else contextlib.nullcontext()
            )
            with loop_cm:
                ohctr = 0
                for b in [bb for _ in range(repeat) for bb in range(nseq)]:
                    # main [128, 1024] + extension cols [1024:1280] holding
                    # the NEXT partition's first 4 chunk-cols, so the shifted
                    # operand e[i+s] is a pure free-dim offset. Partition
                    # 127's extension (positions >= 2048) is zero filler:
                    # 32-aligned memset first, ext DMA overwrites rows 96-126.
                    embA = embp.tile([P, NCH * D + EXT], F16, tag="embA")
                    nc.vector.memset(embA[P - 32 : P, NCH * D :], 0.0)
                    nc.sync.dma_start(out=embA[:, 0 : NCH * D], in_=emb16[b])
                    nc.sync.dma_start(
                        out=embA[0 : P - 1, NCH * D :], in_=emb16[b][1:P, 0:EXT]
                    )
                    idxC = embp.tile([P, 2 * KP1 * NCH], F32, tag="idxC")
                    nc.sync.dma_start(out=idxC[:], in_=idxc[b])

                    if dmov:
                        # interleaved per-pair layout: col c*128 + half*64 + d
                        # so matmul lhsT for (c, pair) is one contiguous
                        # [128, 128] slice covering both gaps of the pair
                        vp = []
                        emb3 = embA[:, 0 : NCH * D].rearrange(
                            "p (c d) -> p c d", c=NCH, d=D
                        )
                        for pair in range(2):
                            v2 = embp.tile([P, NCH * 2 * D], F16, tag=f"vp{pair}")
                            v3 = v2[:].rearrange("p (c hd) -> p c hd", c=NCH)
                            for half in range(2):
                                t = 2 * pair + half
                                s = t + 1
                                eng(tt_engine).tensor_tensor(
                                    out=v3[:, :, half * D : (half + 1) * D],
                                    in0=emb3,
                                    in1=embA[:, s * D : s * D + NCH * D].rearrange(
                                        "p (c d) -> p c d", c=NCH, d=D
                                    ),
                                    op=mybir.AluOpType.add,
                                )
                            vp.append(v2)
                    else:
                        vals = []
                        for t in range(KP1):
                            s = t + 1
                            if tt_engine == "none":  # timing probe (wrong result)
                                vals.append(embA)
                                continue
                            if tt_engine == "split":
                                tt_e = nc.vector if t % 2 == 0 else nc.gpsimd
                            else:
                                tt_e = eng(tt_engine)
                            v = embp.tile([P, NCH * D], F16, tag=f"v{t}")
                            tt_e.tensor_tensor(
                                out=v[:],
                                in0=embA[:, 0 : NCH * D],
                                in1=embA[:, s * D : s * D + NCH * D],
                                op=mybir.AluOpType.add,
                            )
                            vals.append(v)

                    def build_oh(dst, c, t):
                        nonlocal ohctr
                        col = t * NCH + c
                        sc = float(0.5 / (L - t - 1))  # folded into one-hot
                        if act_mod is not None:
                            e_ = (
                                "a"
                                if (act_mod and (ohctr + 1) % act_mod == 0)
                                else "v"
                            )
                        else:
                            e_ = oh_pattern[ohctr % len(oh_pattern)]
                        ohctr += 1
                        if e_ in ("a", "h"):
                            tmp = ohp.tile([P, NBINS], F16, tag="ohtmp")
                            if e_ == "h":
                                # |iota - idx| on DVE, Relu on ACT
                                nc.vector.tensor_scalar(
                                    out=tmp[:],
                                    in0=iota_t[:],
                                    scalar1=idxC[:, col : col + 1],
                                    scalar2=0.0,
                                    op0=mybir.AluOpType.subtract,
                                    op1=mybir.AluOpType.abs_max,
                                )
                            else:
                                nc.scalar.activation(
                                    out=tmp[:],
                                    in_=iota_t[:],
                                    func=mybir.ActivationFunctionType.Abs,
                                    bias=idxC[
                                        :, KP1 * NCH + col : KP1 * NCH + col + 1
                                    ],
                                    scale=1.0,
                                )
                            # Relu(sc - sc*|d|) = sc * onehot (exact)
                            nc.scalar.activation(
                                out=dst,
                                in_=tmp[:],
                                func=mybir.ActivationFunctionType.Relu,
                                bias=scb_t[:, t : t + 1] if fold_scale else 1.0,
                                scale=-sc if fold_scale else -1.0,
                            )
                        else:
                            oh_e = nc.gpsimd if e_ == "g" else nc.vector
                            if fold_scale:
                                oh_e.tensor_scalar(
                                    out=dst,
                                    in0=iota_t[:],
                                    scalar1=idxC[:, col : col + 1],
                                    scalar2=sc,
                                    op0=mybir.AluOpType.is_equal,
                                    op1=mybir.AluOpType.mult,
                                )
                            else:
                                oh_e.tensor_scalar(
                                    out=dst,
                                    in0=iota_t[:],
                                    scalar1=idxC[:, col : col + 1],
                                    scalar2=None,
                                    op0=mybir.AluOpType.is_equal,
                                )

                    if dmov:
                        assert colpack and fold_scale
                        pss = [
                            psp.tile(
                                [P, 2 * NBINS], F32, tag=f"pp{i}", space="PSUM",
                                name=f"pp{i}_{b}",
                            )
                            for i in range(2)
                        ]
                        for c in range(NCH):
                            for pair in range(2):
                                oh2 = ohp.tile([P, 2 * NBINS], F16, tag="oh")
                                for half in range(2):
                                    build_oh(
                                        oh2[:, half * NBINS : (half + 1) * NBINS],
                                        c,
                                        2 * pair + half,
                                    )
                                # one [128,128] stationary x [128,800] moving;
                                # good quadrants: (rows 0:64, cols 0:400) and
                                # (rows 64:128, cols 400:800)
                                nc.tensor.matmul(
                                    out=pss[pair][:],
                                    lhsT=vp[pair][:, c * 2 * D : (c + 1) * 2 * D],
                                    rhs=oh2[:],
                                    start=(c == 0),
                                    stop=(c == NCH - 1),
                                )
                    elif colpack:
                        pss = [
                            psp.tile(
                                [P, NBINS], F32, tag=f"pp{i}", space="PSUM",
                                name=f"pp{i}_{b}",
                            )
                            for i in range(2)
                        ]
                        for c in range(NCH):
                            for t in range(KP1):
                                pair, half = divmod(t, 2)
                                oh = ohp.tile([P, NBINS], F16, tag="oh")
                                build_oh(oh[:], c, t)
                                nc.tensor.matmul(
                                    out=pss[pair][half * D : (half + 1) * D, :],
                                    lhsT=vals[t][:, c * D : (c + 1) * D],
                                    rhs=oh[:],
                                    start=(c == 0),
                                    stop=(c == NCH - 1),
                                    tile_position=(0, half * D),
                                )
                    else:
                        pss = [
                            psp.tile(
                                [D, NBINS], F32, tag=f"pt{t}", space="PSUM",
                                name=f"pt{t}_{b}",
                            )
                            for t in range(KP1)
                        ]
                        for c in range(NCH):
                            for t in range(KP1):
                                oh = ohp.tile([P, NBINS], F16, tag="oh")
                                build_oh(oh[:], c, t)
                                nc.tensor.matmul(
                                    out=pss[t][:],
                                    lhsT=vals[t][:, c * D : (c + 1) * D],
                                    rhs=oh[:],
                                    start=(c == 0),
                                    stop=(c == NCH - 1),
                                )
                    if dmov:
                        for pair in range(2):
                            st = outsp.tile([P, NBINS], OUT_DT, tag="st")
                            nc.scalar.mul(
                                out=st[0:D, :], in_=pss[pair][0:D, 0:NBINS], mul=1.0
                            )
                            nc.scalar.mul(
                                out=st[D:P, :],
                                in_=pss[pair][D:P, NBINS : 2 * NBINS],
                                mul=1.0,
                            )
                            eng(out_dma).dma_start(
                                out=hist[b, pair * P : (pair + 1) * P],
                                in_=st[:],
                            )
                    elif colpack:
                        # evacuate both gaps of a pair in one [128, 400]
                        # op + one DMA; scale folded into one-hots, or
                        # applied here per-partition when fold_scale=False
                        for pair in range(2):
                            st = outsp.tile([P, NBINS], OUT_DT, tag="st")
                            if not fold_scale:
                                nc.vector.tensor_scalar(
                                    out=st[:],
                                    in0=pss[pair][:],
                                    scalar1=scb_t[:, KP1 + pair : KP1 + pair + 1],
                                    scalar2=None,
                                    op0=mybir.AluOpType.mult,
                                )
                            elif evac_engine == "vector":
                                nc.vector.tensor_copy(out=st[:], in_=pss[pair][:])
                            else:
                                nc.scalar.mul(out=st[:], in_=pss[pair][:], mul=1.0)
                            eng(out_dma).dma_start(
                                out=hist[b, pair * P : (pair + 1) * P],
                                in_=st[:],
                            )
                    else:
                        for t in range(KP1):
                            st = outsp.tile([D, NBINS], OUT_DT, tag="st")
                            nc.scalar.mul(out=st[:], in_=pss[t][:], mul=1.0)
                            eng(out_dma).dma_start(
                                out=hist[b, t * D : (t + 1) * D], in_=st[:]
                            )

    nc.compile()
    return nc


def host_prep(seq, emb, nseq_total=B):
    """Full-batch host-side input prep (cheap integer/cast work only)."""
    s = np.asarray(seq).astype(np.int64)
    e = np.asarray(emb, dtype=np.float32).astype(np.float16)
    n_b = s.shape[0]
    # position-major: partition p holds positions [16p, 16p+16)
    emb16 = np.ascontiguousarray(e.reshape(n_b, P, NCH * D))
    idx = np.full((n_b, KP1, L), -1.0, np.float32)
    for t in range(KP1):
        n = L - t - 1
        idx[:, t, :n] = (s[:, :n] * 20 + s[:, t + 1 : t + 1 + n]).astype(np.float32)
    # [b, t, 16p+c] -> [b, p, t*16+c]; concat +idx and -idx along cols
    idxp = idx.reshape(n_b, KP1, P, NCH).transpose(0, 2, 1, 3).reshape(
        n_b, P, KP1 * NCH
    )
    idxc = np.ascontiguousarray(np.concatenate([idxp, -idxp], axis=2))
    iota = np.ascontiguousarray(
        np.broadcast_to(np.arange(NBINS, dtype=np.float16), (P, NBINS))
    )
    sc = np.array([0.5 / (L - t - 1) for t in range(KP1)], np.float32)
    scb = np.empty((P, KP1 + 2), np.float32)
    scb[:, :KP1] = sc
    for pair in range(2):
        scb[:D, KP1 + pair] = sc[2 * pair]
        scb[D:, KP1 + pair] = sc[2 * pair + 1]
    return emb16, idxc, iota, scb


_prog_cache = {}


def get_program(nseq=NSEQ, act_mod=None):
    key = (nseq, act_mod)
    if key not in _prog_cache:
        _prog_cache[key] = build_program(nseq, act_mod)
    return _prog_cache[key]


def make_in_maps(emb16, idxc, iota, scb, nseq=NSEQ, ncores=NCORES):
    in_maps = []
    for ci in range(ncores):
        sl = slice(ci * nseq, (ci + 1) * nseq)
        in_maps.append(
            {
                "emb16": np.ascontiguousarray(emb16[sl]),
                "idxc": np.ascontiguousarray(idxc[sl]),
                "iota": iota,
                "scb": scb,
            }
        )
    return in_maps


def postprocess(hists):
    # hists: [n_b, KP1*D, NBINS] -> [n_b, KP1, 20, 20, D]
    n_b = hists.shape[0]
    return np.ascontiguousarray(
        hists.reshape(n_b, KP1, D, NBINS)
        .transpose(0, 1, 3, 2)
        .reshape(n_b, KP1, 20, 20, D)
    ).astype(np.float32)


def kernel(seq, emb, k):
    assert int(k) == 3, "kernel hardcodes k=3"
    seq = np.asarray(seq)
    emb = np.asarray(emb)
    assert seq.shape == (B, L) and emb.shape == (B, L, D)
    emb16, idxc, iota, scb = host_prep(seq, emb)
    nc = get_program()
    in_maps = make_in_maps(emb16, idxc, iota, scb)
    res = run_bass_kernel_spmd(nc, in_maps, list(range(NCORES)))
    hists = np.concatenate(
        [res.results[ci]["hist"] for ci in range(NCORES)], axis=0
    )
    return postprocess(hists)

